# revision 34
# baseline (speedup 1.0000x reference)
"""Trainium2 Bass kernel for nn_MultiHeadDecoder (sparse attention decoder).

Math (reference, B=1, N=50000, D=512):
    concat    = W_context[0] @ [l; context]                  (1, D)
    g_context = W_graph[0]   @ [g; concat]                   (1, D)
    Q         = g_context @ W_query                          (1, D)
    K         = q @ W_key                                    (N, D)
    compat    = 10 * tanh(norm * Q @ K^T), masked -> -inf    (N,)
    outputs: q[argmax], softmax[argmax], log_softmax[argmax], concat, mask, argmax

Key algebraic optimization: scores = (q @ W_key) @ Q^T == q @ (W_key @ Q^T).
W_key @ Q^T is a tiny (D,D)@(D,) matvec done on host, so the device never
materializes K -- it streams q once and does a 50000x512 matvec + tanh +
masked softmax reductions. This makes the kernel HBM-bound, not GEMM-bound.

Device mapping (per core, 6272-node shard, fp16 streaming):
  - q is host-transposed to dim-major [D, NS] fp16; 4 dim-chunks of 128
    rows DMA in as [128, NS] tiles (12.5 KB contiguous per partition).
  - TensorE: per chunk k, v_k [128,1] is the stationary operand; 13
    matmuls of N=512 nodes write partial scores [1, 512] into row j of
    PSUM bank k. DVE sums the 4 banks -> scores [13, 512] f32.
  - ACT/DVE epilogue: tanh, masked max / sum-exp / first-argmax per
    partition row -> stats [13, 3] back to host.
  - Host: O(100) combine across 8 cores, exact argmax-score refinement.
"""

import math

import numpy as np

import concourse.bass as bass
import concourse.tile as tile
from concourse import bacc, mybir
from concourse.bass_utils import run_bass_kernel_spmd

N_CORES = 8
N = 50000
D = 512
P = 128                      # SBUF partitions
NT = 49                      # 128-node tiles per core
NS = P * NT                  # 6272 nodes per core shard
N_PAD = N_CORES * NS         # 50176
KC = D // P                  # 4 contraction chunks of 128 dims
NT_PE = 33                   # node tiles handled by TensorE (dim-major)
NT_DV = NT - NT_PE           # 16 columns handled by VectorE (node-major)
SPLIT = NT_PE * P            # 4224: first node of the DVE region
NV = NT_DV * P               # 2048 DVE-region nodes
FV = 4                       # consecutive nodes per partition (4 KB runs)
TV = NV // (P * FV)          # 4 big DVE tiles of 512 nodes
NORM = 1.0 / math.sqrt(D)
MASK_NEG = -1000.0           # additive mask; real scores are in [-10, 10]
BIG_IDX = 1.0e30
V_SCALE = 256.0              # keep v well inside fp16 normal range

_prog_cache = {}


def _build_program():
    f32 = mybir.dt.float32
    f16 = mybir.dt.float16
    nc = bacc.Bacc("TRN2", target_bir_lowering=False)

    qst = nc.dram_tensor("qst", [D, SPLIT], f16, kind="ExternalInput")
    qnm = nc.dram_tensor("qnm", [NV, D], f16, kind="ExternalInput")
    vt = nc.dram_tensor("vt", [P, KC], f16, kind="ExternalInput")
    vb = nc.dram_tensor("vb", [1, D], f16, kind="ExternalInput")
    madd = nc.dram_tensor("madd", [P, NT], f32, kind="ExternalInput")
    mbin = nc.dram_tensor("mbin", [P, NT], f32, kind="ExternalInput")
    idxc = nc.dram_tensor("idxc", [P, NT], f32, kind="ExternalInput")
    stats = nc.dram_tensor("stats", [P, 3], f32, kind="ExternalOutput")

    with tile.TileContext(nc) as tc:
        with (
            tc.tile_pool(name="const", bufs=1) as constp,
            tc.tile_pool(name="qp", bufs=1) as qp,
            tc.tile_pool(name="acc", bufs=1) as accp,
            tc.tile_pool(name="ps", bufs=1, space="PSUM") as psp,
        ):
            vtt = constp.tile([P, KC], f16)
            nc.sync.dma_start(out=vtt[:], in_=vt[:])
            vbt = constp.tile([P, D], f16)
            nc.sync.dma_start(out=vbt[:], in_=vb[:].to_broadcast([P, D]))

            # q DMAs next: they pace the kernel. PE's dim-major chunks are
            # split into node-quarters so the PE can start on a quarter as
            # soon as it lands; DVE's node-major tiles (4 KB contiguous per
            # partition) are interleaved so DVE also starts early.
            qbounds = [0, 4, 12, 22, NT_PE]
            # qnm rows: node SPLIT + t*512 + p*4 + f -> [p, t, (f d)]
            qnmv = qnm[:].rearrange("(t p f) d -> p t (f d)", p=P, f=FV)
            ckq = {}
            qv = []
            for qi in range(len(qbounds) - 1):
                lo, hi = qbounds[qi], qbounds[qi + 1]
                for k in range(KC):
                    tle = qp.tile([P, (hi - lo) * P], f16, tag=f"ck{k}q{qi}")
                    nc.sync.dma_start(
                        out=tle[:],
                        in_=qst[k * P : (k + 1) * P, lo * P : hi * P],
                    )
                    ckq[(k, qi)] = tle
                if qi < TV:
                    qvt = qp.tile([P, FV * D], f16, tag=f"qv{qi}")
                    nc.sync.dma_start(out=qvt[:], in_=qnmv[:, qi, :])
                    qv.append(qvt)
            maddt = constp.tile([P, NT], f32)
            nc.sync.dma_start(out=maddt[:], in_=madd[:])
            mbint = constp.tile([P, NT], f32)
            nc.sync.dma_start(out=mbint[:], in_=mbin[:])
            idxt = constp.tile([P, NT], f32)
            nc.sync.dma_start(out=idxt[:], in_=idxc[:])
            bigt = constp.tile([P, NT], f32)
            nc.vector.memset(bigt[:], BIG_IDX)

            # partial scores: PSUM bank k holds chunk k's [128 nodes, NT_PE]
            # column dots; every (k, t) location is written by exactly one
            # matmul (start=stop=True) so groups never interleave in a bank.
            # DVE columns accumulate straight into `scores` via fused STT.
            scores = accp.tile([P, NT], f32)
            pss = []
            for k in range(KC):
                ps = psp.tile([P, NT_PE], f32, tag=f"ps{k}")
                pss.append(ps)
            for qi in range(len(qbounds) - 1):
                lo, hi = qbounds[qi], qbounds[qi + 1]
                for k in range(KC):
                    tle = ckq[(k, qi)]
                    for t in range(lo, hi):
                        nc.tensor.matmul(
                            pss[k][:, t : t + 1],
                            tle[:, (t - lo) * P : (t - lo + 1) * P],
                            vtt[:, k : k + 1],
                            start=True,
                            stop=True,
                        )
                if qi < TV:
                    for f in range(FV):
                        col = NT_PE + qi * FV + f
                        prod = accp.tile([P, D], f16, tag="prod")
                        nc.vector.scalar_tensor_tensor(
                            out=prod[:],
                            in0=qv[qi][:, f * D : (f + 1) * D],
                            scalar=1.0,
                            in1=vbt[:],
                            op0=mybir.AluOpType.mult,
                            op1=mybir.AluOpType.mult,
                            accum_out=scores[:, col : col + 1],
                        )

            # combine the 4 chunk banks -> scores[:, :NT_PE] f32 in SBUF
            # (an op may read at most one PSUM input: copy, then 3 adds)
            nc.scalar.copy(out=scores[:, 0:NT_PE], in_=pss[0][:])
            nc.vector.tensor_add(scores[:, 0:NT_PE], scores[:, 0:NT_PE], pss[1][:])
            nc.vector.tensor_add(scores[:, 0:NT_PE], scores[:, 0:NT_PE], pss[2][:])
            nc.vector.tensor_add(scores[:, 0:NT_PE], scores[:, 0:NT_PE], pss[3][:])

            # tanh_t = tanh(norm * s);  cm = 10*tanh_t + madd (masked score)
            tanh_t = accp.tile([P, NT], f32)
            nc.scalar.activation(
                out=tanh_t[:], in_=scores[:],
                func=mybir.ActivationFunctionType.Tanh,
                scale=float(NORM / V_SCALE),
            )
            cm = accp.tile([P, NT], f32)
            nc.vector.scalar_tensor_tensor(
                out=cm[:], in0=tanh_t[:], scalar=10.0, in1=maddt[:],
                op0=mybir.AluOpType.mult, op1=mybir.AluOpType.add,
            )

            st = accp.tile([P, 3], f32)
            nc.vector.reduce_max(
                out=st[:, 0:1], in_=cm[:], axis=mybir.AxisListType.X
            )
            # e = exp(10*tanh_t); masked row-sum via (e*1.0)*mbin fused accum
            e_t = accp.tile([P, NT], f32)
            nc.scalar.activation(
                out=e_t[:], in_=tanh_t[:],
                func=mybir.ActivationFunctionType.Exp, scale=10.0,
            )
            escr = accp.tile([P, NT], f32)
            nc.vector.scalar_tensor_tensor(
                out=escr[:], in0=e_t[:], scalar=1.0, in1=mbint[:],
                op0=mybir.AluOpType.mult, op1=mybir.AluOpType.mult,
                accum_out=st[:, 1:2],
            )
            # argmax: first (lowest local index) column hitting the row max
            iseq = accp.tile([P, NT], mybir.dt.int32)
            nc.vector.tensor_scalar(
                out=iseq[:], in0=cm[:], scalar1=st[:, 0:1], scalar2=None,
                op0=mybir.AluOpType.is_ge,
            )
            idxsel = accp.tile([P, NT], f32)
            nc.vector.select(idxsel[:], iseq[:], idxt[:], bigt[:])
            nc.vector.tensor_reduce(
                out=st[:, 2:3], in_=idxsel[:],
                op=mybir.AluOpType.min, axis=mybir.AxisListType.X,
            )

            nc.sync.dma_start(out=stats[:], in_=st[:])

    nc.compile()
    return nc


def _get_program():
    if "nc" not in _prog_cache:
        _prog_cache["nc"] = _build_program()
    return _prog_cache["nc"]


def _host_small_math(l, context, g, W_context, W_graph, W_query, W_key):
    """concat (f32, matches reference op order) and v: scores = q @ v."""
    lf = l.reshape(-1, D).astype(np.float32)
    cf = context.reshape(-1, D).astype(np.float32)
    gf = g.reshape(-1, D).astype(np.float32)
    Wc = np.asarray(W_context, np.float32)[0]   # (1,2)
    Wg = np.asarray(W_graph, np.float32)[0]     # (1,2)

    lc = np.concatenate([lf, cf], axis=0)       # (2, D)
    concat = (Wc @ lc).reshape(1, 1, D)         # f32, same as reference

    concat64 = concat.reshape(D).astype(np.float64)
    gctx = Wg[0, 0].astype(np.float64) * gf.reshape(D).astype(np.float64) \
        + Wg[0, 1].astype(np.float64) * concat64
    Q = gctx @ np.asarray(W_query, np.float64)          # (D,)
    v = np.asarray(W_key, np.float64) @ Q               # (D,)
    return concat, v


def kernel(q, l, context, g, mask, is_random, random_net,
           W_context, W_graph, W_query, W_key):
    q = np.asarray(q)
    mask = np.asarray(mask)
    concat, v = _host_small_math(
        l, context, g, W_context, W_graph, W_query, W_key
    )

    # ---- shard inputs -----------------------------------------------------
    q16pad = np.zeros((N_PAD, D), dtype=np.float16)
    q16pad[:N] = q.reshape(N, D).astype(np.float16)

    mask_valid = np.zeros(N_PAD, dtype=bool)
    mask_valid[:N] = mask.reshape(N) > 0

    v16 = (v * V_SCALE).astype(np.float16)
    vt = np.ascontiguousarray(v16.reshape(KC, P).T)  # vt[p,k] = v[k*128+p]
    vb = v16.reshape(1, D)

    # device (partition p, column col) -> local node id
    pp = np.arange(P)
    nodes_for = np.empty((P, NT), np.int64)
    for col in range(NT_PE):
        nodes_for[:, col] = col * P + pp
    for t in range(TV):
        for f in range(FV):
            nodes_for[:, NT_PE + t * FV + f] = SPLIT + t * (P * FV) + pp * FV + f

    def shard_map(arr, c):
        return np.ascontiguousarray(
            arr[c * NS : (c + 1) * NS][nodes_for].astype(np.float32)
        )

    madd_all = np.where(mask_valid, 0.0, MASK_NEG).astype(np.float32)
    mbin_all = mask_valid.astype(np.float32)
    idxc = np.ascontiguousarray(nodes_for.astype(np.float32))

    in_maps = []
    for c in range(N_CORES):
        in_maps.append({
            "qst": np.ascontiguousarray(
                q16pad[c * NS : c * NS + SPLIT].T
            ),
            "qnm": q16pad[c * NS + SPLIT : (c + 1) * NS],
            "vt": vt,
            "vb": vb,
            "madd": shard_map(madd_all, c),
            "mbin": shard_map(mbin_all, c),
            "idxc": idxc,
        })

    # ---- run on 8 NeuronCores --------------------------------------------
    nc = _get_program()
    res = run_bass_kernel_spmd(nc, in_maps, core_ids=list(range(N_CORES)))
    _prog_cache["last_results"] = res
    stats = np.stack([res.results[c]["stats"] for c in range(N_CORES)])
    # stats: [8, P, 3] = (row max, row sum-exp, row argmax local idx)

    # ---- host combine (O(100)) -------------------------------------------
    pmax = stats[:, :, 0].astype(np.float64)
    psum = stats[:, :, 1].astype(np.float64)
    pidx = stats[:, :, 2]

    total = psum.sum()
    allmax = pmax.max()
    cand = np.argwhere(pmax == allmax)
    node = min(int(c) * NS + int(pidx[c, r]) for c, r in cand)

    q64 = q.reshape(N, D).astype(np.float64)
    v64 = v.astype(np.float64)

    def exact_score(i):
        return 10.0 * math.tanh(NORM * float(q64[i] @ v64))

    if int(np.asarray(is_random)):
        idx = int(np.asarray(random_net).reshape(-1)[0])
        c_val = exact_score(idx)
        if not mask_valid[idx]:
            attn = 0.0
            log_attn = -np.inf
        else:
            attn = math.exp(c_val) / total
            log_attn = c_val - math.log(total)
        max_indx = np.asarray(random_net).reshape(1, 1).astype(np.int32)
    else:
        idx = node
        # refine: exact argmax score on host; swap its term inside the
        # device-accumulated sum-of-exp (kills the fp16 error on the
        # numerator; the denominator residual is a softmax-weighted
        # average of independent fp16 errors, ~4e-5)
        c_exact = exact_score(idx)
        total = total - math.exp(allmax) + math.exp(c_exact)
        attn = math.exp(c_exact) / total
        log_attn = c_exact - math.log(total)
        max_indx = np.array([[idx]], dtype=np.int32)

    q_max = q.reshape(N, D)[idx].reshape(1, 1, D).astype(np.float32)
    attn_max = np.array([[attn]], dtype=np.float32)
    log_attn_max = np.array([[log_attn]], dtype=np.float32)
    mask_copy = mask.reshape(1, N).astype(np.int32)

    return (q_max, attn_max, log_attn_max, concat, mask_copy, max_indx)


# revision 35
# speedup vs baseline: 1.1526x; 1.1526x over previous
"""Trainium2 Bass kernel for nn_MultiHeadDecoder (sparse attention decoder).

Math (reference, B=1, N=50000, D=512):
    concat    = W_context[0] @ [l; context]                  (1, D)
    g_context = W_graph[0]   @ [g; concat]                   (1, D)
    Q         = g_context @ W_query                          (1, D)
    K         = q @ W_key                                    (N, D)
    compat    = 10 * tanh(norm * Q @ K^T), masked -> -inf    (N,)
    outputs: q[argmax], softmax[argmax], log_softmax[argmax], concat, mask, argmax

Key algebraic optimization: scores = (q @ W_key) @ Q^T == q @ (W_key @ Q^T).
W_key @ Q^T is a tiny (D,D)@(D,) matvec done on host, so the device never
materializes K -- it streams q once and does a 50000x512 matvec + tanh +
masked softmax reductions. This makes the kernel HBM-bound, not GEMM-bound.

Device mapping (per core, 6272-node shard, fp16 streaming):
  - q is host-transposed to dim-major [D, NS] fp16; 4 dim-chunks of 128
    rows DMA in as [128, NS] tiles (12.5 KB contiguous per partition).
  - TensorE: per chunk k, v_k [128,1] is the stationary operand; 13
    matmuls of N=512 nodes write partial scores [1, 512] into row j of
    PSUM bank k. DVE sums the 4 banks -> scores [13, 512] f32.
  - ACT/DVE epilogue: tanh, masked max / sum-exp / first-argmax per
    partition row -> stats [13, 3] back to host.
  - Host: O(100) combine across 8 cores, exact argmax-score refinement.
"""

import math

import numpy as np

import concourse.bass as bass
import concourse.tile as tile
from concourse import bacc, mybir
from concourse.bass_utils import run_bass_kernel_spmd

N_CORES = 8
N = 50000
D = 512
P = 128                      # SBUF partitions
NT = 49                      # 128-node tiles per core
NS = P * NT                  # 6272 nodes per core shard
N_PAD = N_CORES * NS         # 50176
KC = D // P                  # 4 contraction chunks of 128 dims
NORM = 1.0 / math.sqrt(D)
MASK_NEG = -1000.0           # additive mask; real scores are in [-10, 10]
BIG_IDX = 1.0e30
V_SCALE = 256.0              # keep v well inside fp16 normal range

_prog_cache = {}


def _build_program():
    f32 = mybir.dt.float32
    f16 = mybir.dt.float16
    nc = bacc.Bacc("TRN2", target_bir_lowering=False)

    qst = nc.dram_tensor("qst", [D, NS], f16, kind="ExternalInput")
    vt = nc.dram_tensor("vt", [P, KC], f16, kind="ExternalInput")
    madd = nc.dram_tensor("madd", [P, NT], f32, kind="ExternalInput")
    mbin = nc.dram_tensor("mbin", [P, NT], f32, kind="ExternalInput")
    idxc = nc.dram_tensor("idxc", [P, NT], f32, kind="ExternalInput")
    stats = nc.dram_tensor("stats", [P, 3], f32, kind="ExternalOutput")

    with tile.TileContext(nc) as tc:
        with (
            tc.tile_pool(name="const", bufs=1) as constp,
            tc.tile_pool(name="qp", bufs=1) as qp,
            tc.tile_pool(name="acc", bufs=1) as accp,
            tc.tile_pool(name="ps", bufs=1, space="PSUM") as psp,
        ):
            vtt = constp.tile([P, KC], f16)
            nc.sync.dma_start(out=vtt[:], in_=vt[:])

            # q-chunk DMAs next: they pace the kernel. Each 128-dim chunk
            # is split into 4 node-quarters so the PE can start on a
            # quarter as soon as it lands (3-3.3 KB contiguous runs per
            # partition keep HBM efficiency near peak).
            qbounds = [0, 4, 13, 25, 37, NT]
            ckq = {}
            for k in range(KC):
                for qi in range(len(qbounds) - 1):
                    lo, hi = qbounds[qi], qbounds[qi + 1]
                    tle = qp.tile([P, (hi - lo) * P], f16, tag=f"ck{k}q{qi}")
                    nc.sync.dma_start(
                        out=tle[:],
                        in_=qst[k * P : (k + 1) * P, lo * P : hi * P],
                    )
                    ckq[(k, qi)] = tle
            maddt = constp.tile([P, NT], f32)
            nc.sync.dma_start(out=maddt[:], in_=madd[:])
            mbint = constp.tile([P, NT], f32)
            nc.sync.dma_start(out=mbint[:], in_=mbin[:])
            idxt = constp.tile([P, NT], f32)
            nc.sync.dma_start(out=idxt[:], in_=idxc[:])
            bigt = constp.tile([P, NT], f32)
            nc.vector.memset(bigt[:], BIG_IDX)

            # partial scores: PSUM bank k holds chunk k's [128 nodes, NT]
            # column dots; every (k, t) location is written by exactly one
            # matmul (start=stop=True) so groups never interleave in a bank
            pss = []
            for k in range(KC):
                ps = psp.tile([P, NT], f32, tag=f"ps{k}")
                pss.append(ps)
            for k in range(KC):
                for qi in range(len(qbounds) - 1):
                    lo, hi = qbounds[qi], qbounds[qi + 1]
                    tle = ckq[(k, qi)]
                    for t in range(lo, hi):
                        nc.tensor.matmul(
                            pss[k][:, t : t + 1],
                            tle[:, (t - lo) * P : (t - lo + 1) * P],
                            vtt[:, k : k + 1],
                            start=True,
                            stop=True,
                        )

            # combine the 4 chunk banks -> scores [P, NT] f32 in SBUF
            # (an op may read at most one PSUM input: copy, then 3 adds)
            scores = accp.tile([P, NT], f32)
            nc.scalar.copy(out=scores[:], in_=pss[0][:])
            nc.vector.tensor_add(scores[:], scores[:], pss[1][:])
            nc.vector.tensor_add(scores[:], scores[:], pss[2][:])
            nc.vector.tensor_add(scores[:], scores[:], pss[3][:])

            # tanh_t = tanh(norm * s);  cm = 10*tanh_t + madd (masked score)
            tanh_t = accp.tile([P, NT], f32)
            nc.scalar.activation(
                out=tanh_t[:], in_=scores[:],
                func=mybir.ActivationFunctionType.Tanh,
                scale=float(NORM / V_SCALE),
            )
            cm = accp.tile([P, NT], f32)
            nc.vector.scalar_tensor_tensor(
                out=cm[:], in0=tanh_t[:], scalar=10.0, in1=maddt[:],
                op0=mybir.AluOpType.mult, op1=mybir.AluOpType.add,
            )

            st = accp.tile([P, 3], f32)
            nc.vector.reduce_max(
                out=st[:, 0:1], in_=cm[:], axis=mybir.AxisListType.X
            )
            # e = exp(10*tanh_t); masked row-sum via (e*1.0)*mbin fused accum
            e_t = accp.tile([P, NT], f32)
            nc.scalar.activation(
                out=e_t[:], in_=tanh_t[:],
                func=mybir.ActivationFunctionType.Exp, scale=10.0,
            )
            escr = accp.tile([P, NT], f32)
            nc.vector.scalar_tensor_tensor(
                out=escr[:], in0=e_t[:], scalar=1.0, in1=mbint[:],
                op0=mybir.AluOpType.mult, op1=mybir.AluOpType.mult,
                accum_out=st[:, 1:2],
            )
            # argmax: first (lowest local index) column hitting the row max
            iseq = accp.tile([P, NT], mybir.dt.int32)
            nc.vector.tensor_scalar(
                out=iseq[:], in0=cm[:], scalar1=st[:, 0:1], scalar2=None,
                op0=mybir.AluOpType.is_ge,
            )
            idxsel = accp.tile([P, NT], f32)
            nc.vector.select(idxsel[:], iseq[:], idxt[:], bigt[:])
            nc.vector.tensor_reduce(
                out=st[:, 2:3], in_=idxsel[:],
                op=mybir.AluOpType.min, axis=mybir.AxisListType.X,
            )

            nc.sync.dma_start(out=stats[:], in_=st[:])

    nc.compile()
    return nc


def _get_program():
    if "nc" not in _prog_cache:
        _prog_cache["nc"] = _build_program()
    return _prog_cache["nc"]


def _host_small_math(l, context, g, W_context, W_graph, W_query, W_key):
    """concat (f32, matches reference op order) and v: scores = q @ v."""
    lf = l.reshape(-1, D).astype(np.float32)
    cf = context.reshape(-1, D).astype(np.float32)
    gf = g.reshape(-1, D).astype(np.float32)
    Wc = np.asarray(W_context, np.float32)[0]   # (1,2)
    Wg = np.asarray(W_graph, np.float32)[0]     # (1,2)

    lc = np.concatenate([lf, cf], axis=0)       # (2, D)
    concat = (Wc @ lc).reshape(1, 1, D)         # f32, same as reference

    concat64 = concat.reshape(D).astype(np.float64)
    gctx = Wg[0, 0].astype(np.float64) * gf.reshape(D).astype(np.float64) \
        + Wg[0, 1].astype(np.float64) * concat64
    Q = gctx @ np.asarray(W_query, np.float64)          # (D,)
    v = np.asarray(W_key, np.float64) @ Q               # (D,)
    return concat, v


def kernel(q, l, context, g, mask, is_random, random_net,
           W_context, W_graph, W_query, W_key):
    q = np.asarray(q)
    mask = np.asarray(mask)
    concat, v = _host_small_math(
        l, context, g, W_context, W_graph, W_query, W_key
    )

    # ---- shard inputs -----------------------------------------------------
    # dim-major fp16 q: per-core [D, NS] block -> long per-partition DMA runs
    qpadT = np.zeros((D, N_PAD), dtype=np.float16)
    qpadT[:, :N] = q.reshape(N, D).astype(np.float16).T

    mask_valid = np.zeros(N_PAD, dtype=bool)
    mask_valid[:N] = mask.reshape(N) > 0

    vt = np.ascontiguousarray(
        (v * V_SCALE).astype(np.float16).reshape(KC, P).T
    )  # vt[p, k] = v_scaled[k*128 + p]

    def shard_pt(arr, c):
        # [NS] shard -> [P, NT] with [p, t] = local node t*128+p
        return np.ascontiguousarray(
            arr[c * NS : (c + 1) * NS].reshape(NT, P).T
        )

    madd_all = np.where(mask_valid, 0.0, MASK_NEG).astype(np.float32)
    mbin_all = mask_valid.astype(np.float32)
    idx_all = np.arange(N_PAD, dtype=np.float32) % NS  # local index

    in_maps = []
    for c in range(N_CORES):
        in_maps.append({
            "qst": np.ascontiguousarray(qpadT[:, c * NS : (c + 1) * NS]),
            "vt": vt,
            "madd": shard_pt(madd_all, c),
            "mbin": shard_pt(mbin_all, c),
            "idxc": shard_pt(idx_all, c),
        })

    # ---- run on 8 NeuronCores --------------------------------------------
    nc = _get_program()
    res = run_bass_kernel_spmd(nc, in_maps, core_ids=list(range(N_CORES)))
    _prog_cache["last_results"] = res
    stats = np.stack([res.results[c]["stats"] for c in range(N_CORES)])
    # stats: [8, P, 3] = (row max, row sum-exp, row argmax local idx)

    # ---- host combine (O(100)) -------------------------------------------
    pmax = stats[:, :, 0].astype(np.float64)
    psum = stats[:, :, 1].astype(np.float64)
    pidx = stats[:, :, 2]

    total = psum.sum()
    allmax = pmax.max()
    cand = np.argwhere(pmax == allmax)
    node = min(int(c) * NS + int(pidx[c, r]) for c, r in cand)

    q64 = q.reshape(N, D).astype(np.float64)
    v64 = v.astype(np.float64)

    def exact_score(i):
        return 10.0 * math.tanh(NORM * float(q64[i] @ v64))

    if int(np.asarray(is_random)):
        idx = int(np.asarray(random_net).reshape(-1)[0])
        c_val = exact_score(idx)
        if not mask_valid[idx]:
            attn = 0.0
            log_attn = -np.inf
        else:
            attn = math.exp(c_val) / total
            log_attn = c_val - math.log(total)
        max_indx = np.asarray(random_net).reshape(1, 1).astype(np.int32)
    else:
        idx = node
        # refine: exact argmax score on host; swap its term inside the
        # device-accumulated sum-of-exp (kills the fp16 error on the
        # numerator; the denominator residual is a softmax-weighted
        # average of independent fp16 errors, ~4e-5)
        c_exact = exact_score(idx)
        total = total - math.exp(allmax) + math.exp(c_exact)
        attn = math.exp(c_exact) / total
        log_attn = c_exact - math.log(total)
        max_indx = np.array([[idx]], dtype=np.int32)

    q_max = q.reshape(N, D)[idx].reshape(1, 1, D).astype(np.float32)
    attn_max = np.array([[attn]], dtype=np.float32)
    log_attn_max = np.array([[log_attn]], dtype=np.float32)
    mask_copy = mask.reshape(1, N).astype(np.int32)

    return (q_max, attn_max, log_attn_max, concat, mask_copy, max_indx)


# revision 36
# speedup vs baseline: 1.2015x; 1.0424x over previous
"""Trainium2 Bass kernel for nn_MultiHeadDecoder (sparse attention decoder).

Math (reference, B=1, N=50000, D=512):
    concat    = W_context[0] @ [l; context]                  (1, D)
    g_context = W_graph[0]   @ [g; concat]                   (1, D)
    Q         = g_context @ W_query                          (1, D)
    K         = q @ W_key                                    (N, D)
    compat    = 10 * tanh(norm * Q @ K^T), masked -> -inf    (N,)
    outputs: q[argmax], softmax[argmax], log_softmax[argmax], concat, mask, argmax

Key algebraic optimization: scores = (q @ W_key) @ Q^T == q @ (W_key @ Q^T).
W_key @ Q^T is a tiny (D,D)@(D,) matvec done on host, so the device never
materializes K -- it streams q once and does a 50000x512 matvec + tanh +
masked softmax reductions. This makes the kernel HBM-bound, not GEMM-bound.

Device mapping (per core, 6272-node shard, fp16 streaming):
  - q is host-transposed to dim-major [D, NS] fp16; 4 dim-chunks of 128
    rows DMA in as [128, NS] tiles (12.5 KB contiguous per partition).
  - TensorE: per chunk k, v_k [128,1] is the stationary operand; 13
    matmuls of N=512 nodes write partial scores [1, 512] into row j of
    PSUM bank k. DVE sums the 4 banks -> scores [13, 512] f32.
  - ACT/DVE epilogue: tanh, masked max / sum-exp / first-argmax per
    partition row -> stats [13, 3] back to host.
  - Host: O(100) combine across 8 cores, exact argmax-score refinement.
"""

import math

import numpy as np

import concourse.bass as bass
import concourse.tile as tile
from concourse import bacc, mybir
from concourse.bass_utils import run_bass_kernel_spmd

N_CORES = 8
N = 50000
D = 512
P = 128                      # SBUF partitions
NT = 49                      # 128-node tiles per core
NS = P * NT                  # 6272 nodes per core shard
N_PAD = N_CORES * NS         # 50176
KC = D // P                  # 4 contraction chunks of 128 dims
NORM = 1.0 / math.sqrt(D)
MASK_NEG = -1000.0           # additive mask; real scores are in [-10, 10]
BIG_IDX = 1.0e30
V_SCALE = 256.0              # keep v well inside fp16 normal range

_prog_cache = {}


def _build_program():
    f32 = mybir.dt.float32
    f16 = mybir.dt.float16
    nc = bacc.Bacc("TRN2", target_bir_lowering=False)

    qst = nc.dram_tensor("qst", [D, NS], f16, kind="ExternalInput")
    vt = nc.dram_tensor("vt", [P, KC], f16, kind="ExternalInput")
    madd = nc.dram_tensor("madd", [P, NT], f32, kind="ExternalInput")
    mbin = nc.dram_tensor("mbin", [P, NT], f32, kind="ExternalInput")
    idxc = nc.dram_tensor("idxc", [P, NT], f32, kind="ExternalInput")
    stats = nc.dram_tensor("stats", [P, 3], f32, kind="ExternalOutput")

    with tile.TileContext(nc) as tc:
        with (
            tc.tile_pool(name="const", bufs=1) as constp,
            tc.tile_pool(name="qp", bufs=1) as qp,
            tc.tile_pool(name="acc", bufs=1) as accp,
            tc.tile_pool(name="ps", bufs=1, space="PSUM") as psp,
        ):
            vtt = constp.tile([P, KC], f16)
            nc.sync.dma_start(out=vtt[:], in_=vt[:])

            # q-chunk DMAs next: they pace the kernel. Each 128-dim chunk
            # is split into 4 node-quarters so the PE can start on a
            # quarter as soon as it lands (3-3.3 KB contiguous runs per
            # partition keep HBM efficiency near peak).
            qbounds = [0, 4, 13, 25, 37, NT]
            ckq = {}
            for k in range(KC):
                for qi in range(len(qbounds) - 1):
                    lo, hi = qbounds[qi], qbounds[qi + 1]
                    tle = qp.tile([P, (hi - lo) * P], f16, tag=f"ck{k}q{qi}")
                    nc.sync.dma_start(
                        out=tle[:],
                        in_=qst[k * P : (k + 1) * P, lo * P : hi * P],
                    )
                    ckq[(k, qi)] = tle
            maddt = constp.tile([P, NT], f32)
            nc.sync.dma_start(out=maddt[:], in_=madd[:])
            mbint = constp.tile([P, NT], f32)
            nc.sync.dma_start(out=mbint[:], in_=mbin[:])
            idxt = constp.tile([P, NT], f32)
            nc.sync.dma_start(out=idxt[:], in_=idxc[:])
            bigt = constp.tile([P, NT], f32)
            nc.vector.memset(bigt[:], BIG_IDX)

            # partial scores: PSUM bank k holds chunk k's [128 nodes, NT]
            # column dots; every (k, t) location is written by exactly one
            # matmul (start=stop=True) so groups never interleave in a bank
            pss = []
            for k in range(KC):
                ps = psp.tile([P, NT], f32, tag=f"ps{k}")
                pss.append(ps)
            for k in range(KC):
                for qi in range(len(qbounds) - 1):
                    lo, hi = qbounds[qi], qbounds[qi + 1]
                    tle = ckq[(k, qi)]
                    for t in range(lo, hi):
                        nc.tensor.matmul(
                            pss[k][:, t : t + 1],
                            tle[:, (t - lo) * P : (t - lo + 1) * P],
                            vtt[:, k : k + 1],
                            start=True,
                            stop=True,
                        )

            # combine the 4 chunk banks -> scores [P, NT] f32 in SBUF
            # (an op may read at most one PSUM input: copy, then 3 adds).
            # Done in two column ranges: cols [0, 37) are fully accumulated
            # one PE quarter before the end, so their combine/tanh/exp
            # hides under the final quarter's matmuls; only cols [37, 49)
            # plus the row reductions trail the last matmul.
            scores = accp.tile([P, NT], f32)
            tanh_t = accp.tile([P, NT], f32)
            cm = accp.tile([P, NT], f32)
            e_t = accp.tile([P, NT], f32)
            for lo, hi in ((0, 37), (37, NT)):
                s_ = scores[:, lo:hi]
                nc.scalar.copy(out=s_, in_=pss[0][:, lo:hi])
                nc.vector.tensor_add(s_, s_, pss[1][:, lo:hi])
                nc.vector.tensor_add(s_, s_, pss[2][:, lo:hi])
                nc.vector.tensor_add(s_, s_, pss[3][:, lo:hi])
                # tanh_t = tanh(norm*s); cm = 10*tanh_t + madd; e = exp(10*tanh_t)
                nc.scalar.activation(
                    out=tanh_t[:, lo:hi], in_=s_,
                    func=mybir.ActivationFunctionType.Tanh,
                    scale=float(NORM / V_SCALE),
                )
                nc.vector.scalar_tensor_tensor(
                    out=cm[:, lo:hi], in0=tanh_t[:, lo:hi], scalar=10.0,
                    in1=maddt[:, lo:hi],
                    op0=mybir.AluOpType.mult, op1=mybir.AluOpType.add,
                )
                nc.scalar.activation(
                    out=e_t[:, lo:hi], in_=tanh_t[:, lo:hi],
                    func=mybir.ActivationFunctionType.Exp, scale=10.0,
                )

            st = accp.tile([P, 3], f32)
            nc.vector.reduce_max(
                out=st[:, 0:1], in_=cm[:], axis=mybir.AxisListType.X
            )
            escr = accp.tile([P, NT], f32)
            nc.vector.scalar_tensor_tensor(
                out=escr[:], in0=e_t[:], scalar=1.0, in1=mbint[:],
                op0=mybir.AluOpType.mult, op1=mybir.AluOpType.mult,
                accum_out=st[:, 1:2],
            )
            # argmax: first (lowest local index) column hitting the row max
            iseq = accp.tile([P, NT], mybir.dt.int32)
            nc.vector.tensor_scalar(
                out=iseq[:], in0=cm[:], scalar1=st[:, 0:1], scalar2=None,
                op0=mybir.AluOpType.is_ge,
            )
            idxsel = accp.tile([P, NT], f32)
            nc.vector.select(idxsel[:], iseq[:], idxt[:], bigt[:])
            nc.vector.tensor_reduce(
                out=st[:, 2:3], in_=idxsel[:],
                op=mybir.AluOpType.min, axis=mybir.AxisListType.X,
            )

            nc.sync.dma_start(out=stats[:], in_=st[:])

    nc.compile()
    return nc


def _get_program():
    if "nc" not in _prog_cache:
        _prog_cache["nc"] = _build_program()
    return _prog_cache["nc"]


def _host_small_math(l, context, g, W_context, W_graph, W_query, W_key):
    """concat (f32, matches reference op order) and v: scores = q @ v."""
    lf = l.reshape(-1, D).astype(np.float32)
    cf = context.reshape(-1, D).astype(np.float32)
    gf = g.reshape(-1, D).astype(np.float32)
    Wc = np.asarray(W_context, np.float32)[0]   # (1,2)
    Wg = np.asarray(W_graph, np.float32)[0]     # (1,2)

    lc = np.concatenate([lf, cf], axis=0)       # (2, D)
    concat = (Wc @ lc).reshape(1, 1, D)         # f32, same as reference

    concat64 = concat.reshape(D).astype(np.float64)
    gctx = Wg[0, 0].astype(np.float64) * gf.reshape(D).astype(np.float64) \
        + Wg[0, 1].astype(np.float64) * concat64
    Q = gctx @ np.asarray(W_query, np.float64)          # (D,)
    v = np.asarray(W_key, np.float64) @ Q               # (D,)
    return concat, v


def kernel(q, l, context, g, mask, is_random, random_net,
           W_context, W_graph, W_query, W_key):
    q = np.asarray(q)
    mask = np.asarray(mask)
    concat, v = _host_small_math(
        l, context, g, W_context, W_graph, W_query, W_key
    )

    # ---- shard inputs -----------------------------------------------------
    # dim-major fp16 q: per-core [D, NS] block -> long per-partition DMA runs
    qpadT = np.zeros((D, N_PAD), dtype=np.float16)
    qpadT[:, :N] = q.reshape(N, D).astype(np.float16).T

    mask_valid = np.zeros(N_PAD, dtype=bool)
    mask_valid[:N] = mask.reshape(N) > 0

    vt = np.ascontiguousarray(
        (v * V_SCALE).astype(np.float16).reshape(KC, P).T
    )  # vt[p, k] = v_scaled[k*128 + p]

    def shard_pt(arr, c):
        # [NS] shard -> [P, NT] with [p, t] = local node t*128+p
        return np.ascontiguousarray(
            arr[c * NS : (c + 1) * NS].reshape(NT, P).T
        )

    madd_all = np.where(mask_valid, 0.0, MASK_NEG).astype(np.float32)
    mbin_all = mask_valid.astype(np.float32)
    idx_all = np.arange(N_PAD, dtype=np.float32) % NS  # local index

    in_maps = []
    for c in range(N_CORES):
        in_maps.append({
            "qst": np.ascontiguousarray(qpadT[:, c * NS : (c + 1) * NS]),
            "vt": vt,
            "madd": shard_pt(madd_all, c),
            "mbin": shard_pt(mbin_all, c),
            "idxc": shard_pt(idx_all, c),
        })

    # ---- run on 8 NeuronCores --------------------------------------------
    nc = _get_program()
    res = run_bass_kernel_spmd(nc, in_maps, core_ids=list(range(N_CORES)))
    _prog_cache["last_results"] = res
    stats = np.stack([res.results[c]["stats"] for c in range(N_CORES)])
    # stats: [8, P, 3] = (row max, row sum-exp, row argmax local idx)

    # ---- host combine (O(100)) -------------------------------------------
    pmax = stats[:, :, 0].astype(np.float64)
    psum = stats[:, :, 1].astype(np.float64)
    pidx = stats[:, :, 2]

    total = psum.sum()
    allmax = pmax.max()
    cand = np.argwhere(pmax == allmax)
    node = min(int(c) * NS + int(pidx[c, r]) for c, r in cand)

    q64 = q.reshape(N, D).astype(np.float64)
    v64 = v.astype(np.float64)

    def exact_score(i):
        return 10.0 * math.tanh(NORM * float(q64[i] @ v64))

    if int(np.asarray(is_random)):
        idx = int(np.asarray(random_net).reshape(-1)[0])
        c_val = exact_score(idx)
        if not mask_valid[idx]:
            attn = 0.0
            log_attn = -np.inf
        else:
            attn = math.exp(c_val) / total
            log_attn = c_val - math.log(total)
        max_indx = np.asarray(random_net).reshape(1, 1).astype(np.int32)
    else:
        idx = node
        # refine: exact argmax score on host; swap its term inside the
        # device-accumulated sum-of-exp (kills the fp16 error on the
        # numerator; the denominator residual is a softmax-weighted
        # average of independent fp16 errors, ~4e-5)
        c_exact = exact_score(idx)
        total = total - math.exp(allmax) + math.exp(c_exact)
        attn = math.exp(c_exact) / total
        log_attn = c_exact - math.log(total)
        max_indx = np.array([[idx]], dtype=np.int32)

    q_max = q.reshape(N, D)[idx].reshape(1, 1, D).astype(np.float32)
    attn_max = np.array([[attn]], dtype=np.float32)
    log_attn_max = np.array([[log_attn]], dtype=np.float32)
    mask_copy = mask.reshape(1, N).astype(np.int32)

    return (q_max, attn_max, log_attn_max, concat, mask_copy, max_indx)
